# revision 4
# baseline (speedup 1.0000x reference)
"""Trainium2 Bass kernel for the byte-LSTM autoregressive model.

Problem: B=64, T=8192, D=32 (LSTM state), C=256 output categories.
  xf = x/255 - 0.5
  h_in = shift_right(xf[:,:,None]*W_in[0] + b_in, bos)
  gx = h_in @ Wi + b_lstm                    (gates i,f,g,o)
  (c,h) LSTM scan over T steps with Wh
  out = hs @ W_out + b_out                   [B,T,C]

Sharding: data-parallel over batch, 8 sequences per core, 8 cores.

Algorithm: global Picard iteration, SWEEP-MAJOR over the whole sequence.
The LSTM scan is a fixed point in the h-sequence:

    G      = [h_prev; r3] @ W70            (per 2-seq pair, K=70 matmul)
    T4     = tanh(0.5 G)                   (ACT; sig via tanh identity)
    F      = 0.5 T_f + 0.5   (= sigmoid(f))
    U      = (T_i+1) T_g     (= 2 sig(i) tanh(g);  C = 2c scaled state)
    C_t    = F_t C_{t-1} + U_t             (tensor_tensor_scan)
    m      = tanh(0.5 C)
    H      = (T_o+1) m       (= 2h)

Unlike the tile-sequential baseline (tiles of 512 steps each fully
converged before the next starts -> a 16-tile serial latency chain),
this kernel keeps the WHOLE sequence's H iterate in SBUF, double
buffered (A/B), and runs sweep s over all 16 tiles back to back.  Tile
j's (c,h) carry comes from tile j-1's sweep s-1 state ("stale carry"):
numerically free (carry errors decay ~0.5^t within a tile; validated
max-rel-err 7.9e-3 fp32 at 7 sweeps, same as sequential carries), and
it makes all 16 tiles of a sweep data-independent, so the engines
pipeline across tiles and the kernel runs at ACT (tanh) throughput
instead of dependency-chain latency.  Gate tanh for both quads is a
single [128,2048] ACT op from one PSUM tile per gate pair (IG / FO).
Sweep 1 reads no H at all (h==0 guess -> K=6 matmuls on the r3 rows
only), so the H buffers need no init.  The final sweep fuses the
output projection: after tile j's H is written, out = H @ W_out runs
on PE into PSUM slots recycled from the gate pools, PSUM->SBUF bf16
copies are split across ACT/DVE/Pool, and SP issues the HBM DMA.
b_out is added on host (it is zero for this problem's inputs).

Layouts (per core, BS=8 sequences as 4 pairs / 2 quads):
  RP_A/RP_B (pair p) [70, T+1] bf16: rows 0:32 H of seq 2p (scaled
    H=2h), 32:64 H of seq 2p+1, 64:70 r3 rows (xf_{t-1}, 1, t==0) of
    both seqs.  Column t = [H_{t-1}; r3_t] = matmul rhs for step t.
    Sweep s reads buf[(s-1)%2] cols t0:t0+512 and writes H_t into
    buf[s%2] col t+1; col 0 stays zero.  r3 rows live in BOTH buffers.
  Gate PSUM IGall/FOall [128, 2048] f32: partitions = quad layout
    (pair0 0:64, pair1 64:128); cols [gate_a q0 | gate_b q0 | gate_a
    q1 | gate_b q1] (IG: i|g, FO: f|o), 512 steps each.  8KB/partition
    each -> the two slots fill all 8 PSUM banks; final-sweep outproj
    tiles recycle the same slots by name.
  F/U/c/m [128, 1024] bf16: cols = [quad0 512 | quad1 512].
  ccar A/B [128, 34] bf16: col 2j+q = C carry into tile j quad q.
"""

import os
import numpy as np

import concourse.bass as bass
import concourse.bacc as bacc
import concourse.mybir as mybir
import concourse.tile as tile
from concourse.bass_utils import run_bass_kernel_spmd

F32 = mybir.dt.float32
BF16 = mybir.dt.bfloat16
AX = mybir.ActivationFunctionType
OP = mybir.AluOpType

B, T, D, C = 64, 8192, 32, 256
NCORES = 8
BS = B // NCORES          # batch per core = 8
NPAIR = BS // 2           # 4 pair tiles
S = 512                   # steps per tile

SW = int(os.environ.get("KERNEL_SWEEPS", "7"))
T_RUN = int(os.environ.get("KERNEL_T_OVERRIDE", T))
assert T_RUN % S == 0
NT = T_RUN // S
# bench: repeat the whole compute body R times (R=0: loads only)
REPS = int(os.environ.get("KERNEL_BENCH_REPS", "1"))
# copy-engine split per tile of 8 seqs (a=ACT, d=DVE; Pool can't read PSUM)
COPY_ENG = os.environ.get("KERNEL_COPY_ENG", "adadadad")


def build_bass() -> bass.Bass:
    nc = bacc.Bacc("TRN2", target_bir_lowering=False, debug=False,
                   num_devices=NCORES)

    r3x = nc.dram_tensor("r3x", [6 * NPAIR, T_RUN], BF16, kind="ExternalInput")
    # gate blocks side by side: i, g, f, o -- each [70, 64] (seq0, seq1 cols)
    w70 = nc.dram_tensor("w70", [70, 256], BF16, kind="ExternalInput")
    wout = nc.dram_tensor("wout", [64, C], BF16, kind="ExternalInput")
    out = nc.dram_tensor("out", [BS, T_RUN, C], BF16, kind="ExternalOutput")

    with tile.TileContext(nc) as tc:
        with (
            tc.tile_pool(name="const", bufs=1) as cpool,
            tc.tile_pool(name="gpsum", bufs=1, space="PSUM") as gpool,
            tc.tile_pool(name="work", bufs=2) as wpool,
            tc.tile_pool(name="obuf", bufs=4) as obpool,
        ):
            # ---- persistent SBUF tensors -------------------------------
            RP = [[cpool.tile([70, T_RUN + 1], BF16, name=f"RP{ab}{p}")
                   for p in range(NPAIR)] for ab in range(2)]
            w70sb = cpool.tile([70, 256], BF16)
            woutsb = cpool.tile([64, C], BF16)
            ccar = [cpool.tile([128, 2 * NT + 2], BF16, name=f"ccar{ab}")
                    for ab in range(2)]

            nc.sync.dma_start(w70sb[:, :], w70[:, :])
            nc.sync.dma_start(woutsb[:, :], wout[:, :])
            for ab in range(2):
                for p in range(NPAIR):
                    nc.sync.dma_start(RP[ab][p][64:70, 0:T_RUN],
                                      r3x[6 * p:6 * p + 6, 0:T_RUN])
                    nc.vector.memset(RP[ab][p][0:64, 0:1], 0.0)
                nc.vector.memset(ccar[ab][:, :], 0.0)
            tc.strict_bb_all_engine_barrier()

            for _rep in range(REPS):
                for s in range(1, SW + 1):
                    rpR = RP[(s - 1) % 2]
                    rpW = RP[s % 2]
                    ccR = ccar[(s - 1) % 2]
                    ccW = ccar[s % 2]
                    klo = 64 if s == 1 else 0   # sweep 1: h==0, r3 rows only
                    final = s == SW
                    for j in range(NT):
                        t0 = j * S
                        IGall = gpool.tile([128, 2048], F32, name="IGall",
                                           tag="IGall")
                        FOall = gpool.tile([128, 2048], F32, name="FOall",
                                           tag="FOall")
                        # f,o first: tanh_FO (which gates the scan) starts
                        # as soon as the 8 FO matmuls land
                        for gt, wcol, coff in ((FOall, 128, 0), (FOall, 192, 512),
                                               (IGall, 0, 0), (IGall, 64, 512)):
                            for q in range(2):
                                for pr in range(2):
                                    nc.tensor.matmul(
                                        gt[64 * pr:64 * pr + 64,
                                           1024 * q + coff:1024 * q + coff + 512],
                                        lhsT=w70sb[klo:70, wcol:wcol + 64],
                                        rhs=rpR[2 * q + pr][klo:70, t0:t0 + S],
                                        start=True, stop=True)
                        T_FO = wpool.tile([128, 2048], BF16, name="T_FO")
                        T_IG = wpool.tile([128, 2048], BF16, name="T_IG")
                        nc.scalar.activation(T_FO[:, :], FOall[:, :],
                                             AX.Tanh, scale=0.5)
                        nc.scalar.activation(T_IG[:, :], IGall[:, :],
                                             AX.Tanh, scale=0.5)
                        Fc = wpool.tile([128, 1024], BF16, name="Fc")
                        Uc = wpool.tile([128, 1024], BF16, name="Uc")
                        cq = wpool.tile([128, 1024], BF16, name="cq")
                        mq = wpool.tile([128, 1024], BF16, name="mq")
                        for q in range(2):
                            h = 512 * q
                            nc.vector.tensor_scalar(
                                Fc[:, h:h + 512], T_FO[:, 2 * h:2 * h + 512],
                                0.5, 0.5, op0=OP.mult, op1=OP.add)
                            nc.vector.scalar_tensor_tensor(
                                Uc[:, h:h + 512], T_IG[:, 2 * h:2 * h + 512],
                                1.0, T_IG[:, 2 * h + 512:2 * h + 1024],
                                op0=OP.add, op1=OP.mult)
                            nc.vector.tensor_tensor_scan(
                                cq[:, h:h + 512], Fc[:, h:h + 512],
                                Uc[:, h:h + 512],
                                initial=ccR[:, 2 * j + q:2 * j + q + 1],
                                op0=OP.mult, op1=OP.add)
                        nc.scalar.activation(mq[:, :], cq[:, :],
                                             AX.Tanh, scale=0.5)
                        if not final and j + 1 < NT:
                            for q in range(2):
                                nc.gpsimd.tensor_copy(
                                    ccW[:, 2 * (j + 1) + q:2 * (j + 1) + q + 1],
                                    cq[:, 512 * q + 511:512 * q + 512])
                        for p in range(NPAIR):
                            q, pr = p // 2, p % 2
                            rb = 64 * pr
                            nc.vector.scalar_tensor_tensor(
                                rpW[p][0:64, t0 + 1:t0 + S + 1],
                                T_FO[rb:rb + 64, 1024 * q + 512:1024 * q + 1024],
                                1.0, mq[rb:rb + 64, 512 * q:512 * q + 512],
                                op0=OP.add, op1=OP.mult)

                        if final:
                            # fused output projection for tile j
                            for b in range(BS):
                                P = rpW[b // 2]
                                rb = 32 * (b % 2)
                                po = gpool.tile(
                                    [128, 4 * C], F32, name="po",
                                    tag=("IGall", "FOall")[b % 2])
                                for blk in range(4):
                                    nc.tensor.matmul(
                                        po[:, blk * C:(blk + 1) * C],
                                        lhsT=P[rb:rb + 32,
                                               t0 + 1 + 128 * blk:
                                               t0 + 1 + 128 * (blk + 1)],
                                        rhs=woutsb[rb:rb + 32, :],
                                        start=True, stop=True)
                                osb = obpool.tile([128, 4 * C], BF16,
                                                  name="osb")
                                ce = COPY_ENG[b % len(COPY_ENG)]
                                eng = {"a": nc.scalar, "d": nc.vector,
                                       "p": nc.gpsimd}[ce]
                                if ce == "a":
                                    eng.copy(osb[:, :], po[:, :])
                                else:
                                    eng.tensor_copy(osb[:, :], po[:, :])
                                nc.sync.dma_start(
                                    out[b, t0:t0 + S, :].rearrange(
                                        "(blk t) c -> t blk c", t=128),
                                    osb[:, :].rearrange(
                                        "t (blk c) -> t blk c", c=C))
    nc.compile()
    return nc


def _prep_host(inputs: dict[str, np.ndarray]):
    """Host-side constants shared by all cores (tiny)."""
    f32 = np.float32
    Wi = np.asarray(inputs["Wi"], f32)
    Wh = np.asarray(inputs["Wh"], f32)
    W_in = np.asarray(inputs["W_in"], f32)
    b_in = np.asarray(inputs["b_in"], f32)
    b_lstm = np.asarray(inputs["b_lstm"], f32)
    bos = np.asarray(inputs["bos"], f32)
    W_out = np.asarray(inputs["W_out"], f32)

    # gate order: source (i,f,g,o) -> target col blocks (i, o, g, f);
    # g block doubled so tanh(0.5 * 2g) = tanh(g); scaled state H=2h, C=2c
    perm = np.concatenate([np.arange(0, D), np.arange(3 * D, 4 * D),
                           np.arange(2 * D, 3 * D), np.arange(D, 2 * D)])
    gscale = np.ones(4 * D, f32)
    gscale[2 * D:3 * D] = 2.0

    v = (Wi.T @ W_in[0]).astype(f32)[perm] * gscale
    w = (Wi.T @ b_in + b_lstm).astype(f32)[perm] * gscale
    g0 = (Wi.T @ bos + b_lstm).astype(f32)[perm] * gscale
    wh_eff = (0.5 * Wh[:, perm] * gscale[None, :]).astype(f32)
    w35 = np.concatenate([wh_eff, v[None], w[None], g0[None]], 0).astype(f32)
    # w35 col blocks: 0:32 i, 32:64 o, 64:96 g, 96:128 f
    blocks = {"i": w35[:, 0:D], "o": w35[:, D:2 * D],
              "g": w35[:, 2 * D:3 * D], "f": w35[:, 3 * D:4 * D]}
    w70 = np.zeros((70, 256), f32)
    for gi, gate in enumerate(("i", "g", "f", "o")):
        blk = blocks[gate]                      # [35, 32]
        w70[0:32, 64 * gi:64 * gi + 32] = blk[0:32]
        w70[64:67, 64 * gi:64 * gi + 32] = blk[32:35]
        w70[32:64, 64 * gi + 32:64 * gi + 64] = blk[0:32]
        w70[67:70, 64 * gi + 32:64 * gi + 64] = blk[32:35]
    wout_eff = np.concatenate([0.5 * W_out, 0.5 * W_out], 0).astype(f32)
    return w70, wout_eff


def _bf16(a):
    import ml_dtypes
    return np.asarray(a, ml_dtypes.bfloat16)


def prep_in_maps(inputs: dict[str, np.ndarray]) -> list[dict[str, np.ndarray]]:
    x = np.asarray(inputs["x"])
    assert x.shape == (B, T) and x.dtype == np.int32
    w70, wout_eff = _prep_host(inputs)

    xf = (x.astype(np.float32) / np.float32(255.0) - np.float32(0.5))

    # r3[:, t] = (xf[t-1], 1, 0) for t>=1 ; (0,0,1) at t=0 ; pair layout
    in_maps = []
    for core in range(NCORES):
        xs = xf[core * BS:(core + 1) * BS]           # [BS, T]
        r3x = np.zeros((6 * NPAIR, T_RUN), np.float32)
        for p in range(NPAIR):
            for k in range(2):
                b = 2 * p + k
                r3x[6 * p + 3 * k + 0, 1:] = xs[b, :T_RUN - 1]
                r3x[6 * p + 3 * k + 1, 1:] = 1.0
                r3x[6 * p + 3 * k + 2, 0] = 1.0
        in_maps.append({
            "r3x": _bf16(r3x), "w70": _bf16(w70), "wout": _bf16(wout_eff),
        })
    return in_maps


def kernel(**inputs) -> np.ndarray:
    in_maps = prep_in_maps(inputs)
    nc = build_bass()
    res = run_bass_kernel_spmd(nc, in_maps, core_ids=list(range(NCORES)),
                               trace=TRACE)
    global LAST_RESULTS
    LAST_RESULTS = res
    outs = [res.results[i]["out"].astype(np.float32) for i in range(NCORES)]
    full = np.concatenate(outs, axis=0)
    b_out = np.asarray(inputs["b_out"], np.float32)
    if np.any(b_out):
        full = full + b_out                          # exact, host-side
    return full


TRACE = False           # set True (e.g. from test.py) to capture an NTFF trace
LAST_RESULTS = None     # BassKernelResults of the last kernel() call


if __name__ == "__main__":
    rng = np.random.default_rng(0)
    ins = {
        "x": rng.integers(0, C, size=(B, T), dtype=np.int32),
        "bos": rng.normal(size=(D,)).astype(np.float32) * 0.01,
        "W_in": rng.normal(size=(1, D)).astype(np.float32),
        "b_in": np.zeros((D,), np.float32),
        "Wi": rng.normal(size=(D, 4 * D)).astype(np.float32) / np.sqrt(D),
        "Wh": rng.normal(size=(D, 4 * D)).astype(np.float32) / np.sqrt(D),
        "b_lstm": np.zeros((4 * D,), np.float32),
        "W_out": rng.normal(size=(D, C)).astype(np.float32) / np.sqrt(D),
        "b_out": np.zeros((C,), np.float32),
    }
    o = kernel(**ins)
    print("kernel out", o.shape, o.dtype, float(np.abs(o).max()))
